# revision 14
# baseline (speedup 1.0000x reference)
"""Trainium2 Bass kernel for nn_FeatureRefinement.

Reference computation (bs=16, vl=1024, ql=64, d=1024):
    corr = einsum('bqd,bvd->bqv', Q, V); scores = softmax(corr, axis=1)
    corr_matrix = einsum('bqv,qd->bvd', scores, cor_w)     # cor_w constant over q
    sentence    = WeightedPool(Q)                           # (bs, d)
    sim         = cosine(V, sentence) + log(video_mask)     # (bs, vl)
    features    = concat([V, sim*sim_w, sentence_bcast, corr_matrix], -1)
    out         = relu(features @ mixer_w + mixer_b)

Algebraic restructuring (exact up to fp rounding):
  - softmax over q sums to 1  =>  corr_matrix[b,v,:] == cor_v_w*cor_q_w  (constant)
  - sim_features @ W2  == sim[b,v] * (sim_w.T @ W2)        (rank-1)
  - pooled_query @ W3  == sentence[b] @ W3                 (rank-1 per batch)
  so   out = relu(V @ W1 + sim ⊗ w2v + (sentence@W3 + cor@W4 + bias))
  The only heavy compute is V @ W1 (4x FLOP reduction) plus O(bs*vl*d)
  vector work for the cosine similarity.

Sharding: data-parallel over batch, 2 batches per core on 8 cores. No
collectives; host scatters inputs / gathers outputs.
"""
import sys

sys.path.insert(0, "/opt/trn_rl_repo")

import numpy as np
from contextlib import ExitStack

import concourse.bass as bass
import concourse.tile as tile
from concourse import bacc, mybir
from concourse.bass_utils import run_bass_kernel_spmd
from concourse.masks import make_identity

F32 = mybir.dt.float32
F32R = mybir.dt.float32r
AF = mybir.ActivationFunctionType
AX = mybir.AxisListType

BS, VL, QL, D = 16, 1024, 64, 1024
NCORES = 8
BPC = BS // NCORES          # batches per core
KC = D // 128               # contraction chunks
SS = 512                    # v-rows per super-slab
NSS = VL // SS              # super-slabs per batch
NEG_INF = -1e30

# matmul dtype for the heavy path: float32r runs the PE at full rate
# (1 cyc/row for N>=256) vs 4 cyc/row for plain float32.
MM_DT = F32R


def _build_program():
    nc = bacc.Bacc("TRN2", target_bir_lowering=False, debug=False, num_devices=NCORES)

    v_d = nc.dram_tensor("v", [BPC, VL, D], F32, kind="ExternalInput").ap()
    q_d = nc.dram_tensor("q", [BPC, QL, D], F32, kind="ExternalInput").ap()
    qb_d = nc.dram_tensor("qb", [BPC, QL], F32, kind="ExternalInput").ap()
    vb_d = nc.dram_tensor("vb", [BPC, VL], F32, kind="ExternalInput").ap()
    w1_d = nc.dram_tensor("w1", [D, D], MM_DT, kind="ExternalInput").ap()
    w3_d = nc.dram_tensor("w3", [D, D], MM_DT, kind="ExternalInput").ap()
    w2v_d = nc.dram_tensor("w2v", [1, D], MM_DT, kind="ExternalInput").ap()
    biasc_d = nc.dram_tensor("biasc", [1, D], F32, kind="ExternalInput").ap()
    pw_d = nc.dram_tensor("pw", [D, 1], MM_DT, kind="ExternalInput").ap()
    out_d = nc.dram_tensor("out", [BPC, VL, D], F32, kind="ExternalOutput").ap()

    with tile.TileContext(nc) as tc, ExitStack() as ctx:
        singles = ctx.enter_context(tc.tile_pool(name="singles", bufs=1))
        qstuff = ctx.enter_context(tc.tile_pool(name="qstuff", bufs=1))
        rows = ctx.enter_context(tc.tile_pool(name="rows", bufs=2))
        psA = ctx.enter_context(tc.tile_pool(name="psA", bufs=2, space="PSUM"))
        psOut = ctx.enter_context(tc.tile_pool(name="psOut", bufs=3, space="PSUM"))
        psRow = ctx.enter_context(tc.tile_pool(name="psRow", bufs=3, space="PSUM"))

        ident = singles.tile([128, 128], F32)
        make_identity(nc, ident)

        w1_sb = singles.tile([128, KC, D], MM_DT)
        nc.sync.dma_start(out=w1_sb, in_=w1_d.rearrange("(c p) n -> p c n", p=128))
        pw_sb = singles.tile([128, KC, 1], MM_DT)
        nc.sync.dma_start(out=pw_sb, in_=pw_d.rearrange("(c p) x -> p c x", p=128))
        biasc_sb = singles.tile([1, D], F32)
        nc.sync.dma_start(out=biasc_sb, in_=biasc_d)
        w2v_sb = singles.tile([1, D], MM_DT)
        nc.sync.dma_start(out=w2v_sb, in_=w2v_d)
        ones_f = singles.tile([1, 128], F32)
        nc.vector.memset(ones_f, 1.0)
        ones_r = singles.tile([1, 128], MM_DT)
        nc.vector.tensor_copy(ones_r, ones_f)

        # persistent per-batch small tensors
        sentT2 = qstuff.tile([128, KC, BPC], MM_DT)     # sentence^T chunks, col b
        snsq2 = qstuff.tile([1, BPC], F32)              # clamped ||sentence||^2
        vb_sb = qstuff.tile([1, BPC, VL], F32)          # log(video_mask) rows
        aug_rhs = []                                    # per batch [2, D]

        # ---------------- Phase A: query side (tiny) -----------------
        w3ctx = ExitStack()
        w3pool = w3ctx.enter_context(tc.tile_pool(name="w3pool", bufs=1))
        w3_sb = w3pool.tile([128, KC, D], MM_DT)
        nc.sync.dma_start(out=w3_sb, in_=w3_d.rearrange("(c p) n -> p c n", p=128))

        for b in range(BPC):
            q_sb = qstuff.tile([QL, D], F32, name=f"q{b}")
            nc.sync.dma_start(out=q_sb, in_=q_d[b])
            nc.sync.dma_start(out=vb_sb[:, b, :], in_=vb_d[b : b + 1, :])

            # Q^T chunks [128, KC, QL]
            qt_sb = qstuff.tile([128, KC, QL], MM_DT, name=f"qt{b}")
            for k in range(KC):
                t_ps = psA.tile([128, QL], F32, tag="tps")
                nc.tensor.transpose(t_ps, q_sb[:, k * 128 : (k + 1) * 128],
                                    ident[:QL, :QL])
                nc.vector.tensor_copy(qt_sb[:, k, :], t_ps)

            # alpha^T = (Q @ pool_w)^T : [1, QL]
            al_ps = psRow.tile([1, QL], F32, tag="row")
            for k in range(KC):
                nc.tensor.matmul(al_ps, pw_sb[:, k, :], qt_sb[:, k, :],
                                 start=(k == 0), stop=(k == KC - 1))
            qb_sb = rows.tile([1, QL], F32)
            nc.sync.dma_start(out=qb_sb, in_=qb_d[b : b + 1, :])
            alpha_sb = rows.tile([1, QL], F32)
            nc.vector.tensor_add(alpha_sb, al_ps, qb_sb)

            # softmax over the free dim (1 partition)
            mx = rows.tile([1, 1], F32)
            nc.vector.reduce_max(mx, alpha_sb, axis=AX.X)
            asub = rows.tile([1, QL], F32)
            nc.vector.tensor_scalar_sub(asub, alpha_sb, mx)
            aexp = rows.tile([1, QL], F32)
            asum = rows.tile([1, 1], F32)
            nc.scalar.activation(aexp, asub, AF.Exp, accum_out=asum)
            rsum = rows.tile([1, 1], F32)
            nc.vector.reciprocal(rsum, asum)
            alphas_sb = rows.tile([1, QL], F32)
            nc.vector.tensor_scalar_mul(alphas_sb, aexp, rsum)

            # alphas^T : [QL, 1]
            alT_ps = psRow.tile([QL, 1], F32, tag="row")
            nc.tensor.transpose(alT_ps, alphas_sb, ident[:1, :1])
            alphasT_sb = rows.tile([QL, 1], F32)
            nc.vector.tensor_copy(alphasT_sb, alT_ps)

            # sentence = alphas @ Q : [1, D]  (plain fp32, tiny)
            sent_sb = rows.tile([1, D], F32, tag="sent", bufs=1)
            for h in range(2):
                s_ps = psRow.tile([1, 512], F32, tag="row")
                nc.tensor.matmul(s_ps, alphasT_sb, q_sb[:, h * 512 : (h + 1) * 512],
                                 start=True, stop=True)
                nc.vector.tensor_copy(sent_sb[:, h * 512 : (h + 1) * 512], s_ps)

            # ||sentence||^2 clamped
            strash = rows.tile([1, D], F32, tag="strash", bufs=1)
            ssq = rows.tile([1, 1], F32)
            nc.scalar.activation(strash, sent_sb, AF.Square, accum_out=ssq)
            nc.vector.tensor_scalar_max(snsq2[:, b : b + 1], ssq, 1e-16)

            # sentence^T chunks into column b
            for k in range(KC):
                sT_ps = psRow.tile([128, 1], F32, tag="row")
                nc.tensor.transpose(sT_ps, sent_sb[:, k * 128 : (k + 1) * 128],
                                    ident[:1, :1])
                nc.vector.tensor_copy(sentT2[:, k, b : b + 1], sT_ps)

        # bias row per batch: [1, D] = sentence @ W3 + biasc
        for b in range(BPC):
            bias_b = qstuff.tile([1, D], MM_DT, name=f"bias{b}")
            for h in range(2):
                b_ps = psRow.tile([1, 512], F32, tag="row")
                for k in range(KC):
                    nc.tensor.matmul(b_ps, sentT2[:, k, b : b + 1],
                                     w3_sb[:, k, h * 512 : (h + 1) * 512],
                                     start=(k == 0), stop=(k == KC - 1))
                nc.vector.tensor_add(bias_b[:, h * 512 : (h + 1) * 512], b_ps,
                                     biasc_sb[:, h * 512 : (h + 1) * 512])
            aug_rhs.append(bias_b)

        w3ctx.close()  # release W3's SBUF before the heavy phase

        # ---------------- Phase C: video side (heavy) -----------------
        vload = ctx.enter_context(tc.tile_pool(name="vload", bufs=3))
        trashp = ctx.enter_context(tc.tile_pool(name="trashp", bufs=2))
        vtpool = ctx.enter_context(tc.tile_pool(name="vtpool", bufs=2))
        opool = ctx.enter_context(tc.tile_pool(name="opool", bufs=3))

        for b in range(BPC):
            for ss in range(NSS):
                vt = vtpool.tile([128, KC, SS], MM_DT)
                vnsq_col = rows.tile([128, 4], F32, tag="vnsqc")
                for s4 in range(4):
                    r0 = ss * SS + s4 * 128
                    v_sb = vload.tile([128, D], F32)
                    nc.sync.dma_start(out=v_sb, in_=v_d[b, r0 : r0 + 128, :])
                    # ||v||^2 per row (accumulate over free dim)
                    vtrash = trashp.tile([128, D], F32, tag="vtrash")
                    nc.scalar.activation(vtrash, v_sb, AF.Square,
                                         accum_out=vnsq_col[:, s4 : s4 + 1])
                    # transpose 8 chunks of [128,128] -> vt
                    for g in range(2):
                        t_ps = psA.tile([128, 512], F32, tag="tps")
                        for j in range(4):
                            k = g * 4 + j
                            nc.tensor.transpose(
                                t_ps[:, j * 128 : (j + 1) * 128],
                                v_sb[:, k * 128 : (k + 1) * 128], ident)
                        nc.vector.tensor_copy(
                            vt[:, g * 4 : (g + 1) * 4, s4 * 128 : (s4 + 1) * 128],
                            t_ps.rearrange("p (j c) -> p j c", j=4))

                # vnsq as a row [1, SS] via 4 tiny transposes
                vnr_ps = psRow.tile([1, SS], F32, tag="row")
                for s4 in range(4):
                    nc.tensor.transpose(vnr_ps[:, s4 * 128 : (s4 + 1) * 128],
                                        vnsq_col[:, s4 : s4 + 1], ident)

                # dot = V @ sentence as a row [1, SS]
                dot_ps = psRow.tile([1, SS], F32, tag="row")
                for k in range(KC):
                    nc.tensor.matmul(dot_ps, sentT2[:, k, b : b + 1], vt[:, k, :],
                                     start=(k == 0), stop=(k == KC - 1))

                # sim = dot / sqrt(max(vnsq,eps)*snsq) + log(video_mask)
                t1 = rows.tile([1, SS], F32, tag="t1")
                nc.vector.tensor_scalar(t1, vnr_ps, 1e-16, snsq2[:, b : b + 1],
                                        op0=mybir.AluOpType.max,
                                        op1=mybir.AluOpType.mult)
                t2 = rows.tile([1, SS], F32, tag="t2")
                nc.scalar.activation(t2, t1, AF.Sqrt)
                t3 = rows.tile([1, SS], F32, tag="t3")
                nc.vector.reciprocal(t3, t2)
                t4 = rows.tile([1, SS], F32, tag="t4")
                nc.vector.tensor_mul(t4, dot_ps, t3)
                sim_r = rows.tile([1, SS], MM_DT, tag="simr")
                nc.vector.tensor_add(sim_r, t4,
                                     vb_sb[:, b, ss * SS : (ss + 1) * SS])

                # main matmul + two rank-1 augments + relu
                for i in range(4):
                    out_sb = opool.tile([128, D], F32)
                    for h in range(2):
                        o_ps = psOut.tile([128, 512], F32)
                        for k in range(KC):
                            nc.tensor.matmul(
                                o_ps, vt[:, k, i * 128 : (i + 1) * 128],
                                w1_sb[:, k, h * 512 : (h + 1) * 512],
                                start=(k == 0), stop=False)
                        nc.tensor.matmul(
                            o_ps, sim_r[:, i * 128 : (i + 1) * 128],
                            w2v_sb[:, h * 512 : (h + 1) * 512],
                            start=False, stop=False)
                        nc.tensor.matmul(
                            o_ps, ones_r,
                            aug_rhs[b][:, h * 512 : (h + 1) * 512],
                            start=False, stop=True)
                        nc.scalar.activation(out_sb[:, h * 512 : (h + 1) * 512],
                                             o_ps, AF.Relu)
                    r0 = ss * SS + i * 128
                    nc.sync.dma_start(out=out_d[b, r0 : r0 + 128, :], in_=out_sb)

    nc.compile()
    return nc


_NC = None
_LAST_RESULTS = None


def _get_program():
    global _NC
    if _NC is None:
        _NC = _build_program()
    return _NC


def kernel(video_features, query_features, video_mask, query_mask,
           sim_w, cor_v_w, cor_q_w, pool_w, mixer_w, mixer_b):
    video_features = np.ascontiguousarray(np.asarray(video_features, dtype=np.float32))
    query_features = np.ascontiguousarray(np.asarray(query_features, dtype=np.float32))
    video_mask = np.asarray(video_mask, dtype=np.float32)
    query_mask = np.asarray(query_mask, dtype=np.float32)
    sim_w = np.asarray(sim_w, dtype=np.float32)
    cor_v_w = np.asarray(cor_v_w, dtype=np.float32)
    cor_q_w = np.asarray(cor_q_w, dtype=np.float32)
    pool_w = np.asarray(pool_w, dtype=np.float32)
    mixer_w = np.asarray(mixer_w, dtype=np.float32)
    mixer_b = np.asarray(mixer_b, dtype=np.float32)

    # host-side folds of the weight-only algebra (O(d^2), negligible)
    W1 = np.ascontiguousarray(mixer_w[0:D])
    W2 = mixer_w[D : 2 * D]
    W3 = np.ascontiguousarray(mixer_w[2 * D : 3 * D])
    W4 = mixer_w[3 * D : 4 * D]
    w2v = (sim_w[:, 0] @ W2).astype(np.float32)[None, :]
    cor_vec = (cor_v_w[0] * cor_q_w[0, 0]).astype(np.float32)
    biasc = (cor_vec @ W4 + mixer_b).astype(np.float32)[None, :]
    qbias = ((1.0 - query_mask) * NEG_INF).astype(np.float32)
    vbias = np.log(video_mask + 1e-45).astype(np.float32)

    nc = _get_program()
    in_maps = []
    for c in range(NCORES):
        sl = slice(c * BPC, (c + 1) * BPC)
        in_maps.append({
            "v": video_features[sl],
            "q": query_features[sl],
            "qb": np.ascontiguousarray(qbias[sl]),
            "vb": np.ascontiguousarray(vbias[sl]),
            "w1": W1,
            "w3": W3,
            "w2v": w2v,
            "biasc": biasc,
            "pw": pool_w,
        })
    res = run_bass_kernel_spmd(nc, in_maps, core_ids=list(range(NCORES)))
    global _LAST_RESULTS
    _LAST_RESULTS = res
    out = np.concatenate([res.results[c]["out"] for c in range(NCORES)], axis=0)
    return out.astype(np.float32, copy=False)


# revision 17
# speedup vs baseline: 1.1445x; 1.1445x over previous
"""Trainium2 Bass kernel for nn_FeatureRefinement.

Reference computation (bs=16, vl=1024, ql=64, d=1024):
    corr = einsum('bqd,bvd->bqv', Q, V); scores = softmax(corr, axis=1)
    corr_matrix = einsum('bqv,qd->bvd', scores, cor_w)     # cor_w constant over q
    sentence    = WeightedPool(Q)                           # (bs, d)
    sim         = cosine(V, sentence) + log(video_mask)     # (bs, vl)
    features    = concat([V, sim*sim_w, sentence_bcast, corr_matrix], -1)
    out         = relu(features @ mixer_w + mixer_b)

Algebraic restructuring (exact up to fp rounding):
  - softmax over q sums to 1  =>  corr_matrix[b,v,:] == cor_v_w*cor_q_w  (constant)
  - sim_features @ W2  == sim[b,v] * (sim_w.T @ W2)        (rank-1)
  - pooled_query @ W3  == sentence[b] @ W3                 (rank-1 per batch)
  so   out = relu(V @ W1 + sim ⊗ w2v + 1 ⊗ (sentence@W3 + cor@W4 + bias))
  The only heavy compute is V @ W1 (4x FLOP reduction) plus O(bs*vl*d)
  vector work for the cosine similarity.

Sharding: data-parallel over batch, 2 batches per core on 8 cores. No
collectives; host scatters inputs / gathers outputs.

DMA rings: V slabs (+W3) stream on the SP ring so the PE's transpose
pipeline starts ~immediately; weights/query/outputs ride the ACT ring.
Matmul dtypes: fp32r (hw runs LOW_HIGH 2-pass, ~1.5e-4 rel err) for the
V@W1 path; bf16 for the small rank-1 augment terms.
"""
import sys

sys.path.insert(0, "/opt/trn_rl_repo")

import numpy as np
import ml_dtypes
from contextlib import ExitStack

import concourse.bass as bass
import concourse.tile as tile
from concourse import bacc, mybir
from concourse.bass_utils import run_bass_kernel_spmd
from concourse.masks import make_identity

F32 = mybir.dt.float32
F32R = mybir.dt.float32r
BF16 = mybir.dt.bfloat16
AF = mybir.ActivationFunctionType
AX = mybir.AxisListType

BS, VL, QL, D = 16, 1024, 64, 1024
NCORES = 8
BPC = BS // NCORES          # batches per core
KC = D // 128               # contraction chunks
SS = 512                    # v-rows per super-slab
NSS = VL // SS              # super-slabs per batch
NEG_INF = -1e30

MM_DT = F32R


def _build_program():
    nc = bacc.Bacc("TRN2", target_bir_lowering=False, debug=False, num_devices=NCORES)

    v_d = nc.dram_tensor("v", [BPC, VL, D], MM_DT, kind="ExternalInput").ap()
    q_d = nc.dram_tensor("q", [BPC, QL, D], F32, kind="ExternalInput").ap()
    qb_d = nc.dram_tensor("qb", [BPC, QL], F32, kind="ExternalInput").ap()
    vb_d = nc.dram_tensor("vb", [BPC, VL], F32, kind="ExternalInput").ap()
    w1_d = nc.dram_tensor("w1", [D, D], MM_DT, kind="ExternalInput").ap()
    w3_d = nc.dram_tensor("w3", [D, D], MM_DT, kind="ExternalInput").ap()
    w2v_d = nc.dram_tensor("w2v", [1, D], BF16, kind="ExternalInput").ap()
    biasc_d = nc.dram_tensor("biasc", [1, D], F32, kind="ExternalInput").ap()
    pw_d = nc.dram_tensor("pw", [D, 1], MM_DT, kind="ExternalInput").ap()
    out_d = nc.dram_tensor("out", [BPC, VL, D], F32, kind="ExternalOutput").ap()

    with tile.TileContext(nc) as tc, ExitStack() as ctx:
        singles = ctx.enter_context(tc.tile_pool(name="singles", bufs=1))
        qstuff = ctx.enter_context(tc.tile_pool(name="qstuff", bufs=1))
        rows = ctx.enter_context(tc.tile_pool(name="rows", bufs=2))
        vload = ctx.enter_context(tc.tile_pool(name="vload", bufs=5))
        psA = ctx.enter_context(tc.tile_pool(name="psA", bufs=2, space="PSUM"))
        psOut = ctx.enter_context(tc.tile_pool(name="psOut", bufs=4, space="PSUM"))
        psRow = ctx.enter_context(tc.tile_pool(name="psRow", bufs=2, space="PSUM"))

        # ---- SP-ring DMAs first: V slabs for (b0, ss0), then W3 ----
        pre_v = []
        for s4 in range(4):
            v_sb = vload.tile([128, D], MM_DT, tag="v_sb")
            nc.sync.dma_start(out=v_sb, in_=v_d[0, s4 * 128 : (s4 + 1) * 128, :])
            pre_v.append(v_sb)

        w3ctx = ExitStack()
        w3pool = w3ctx.enter_context(tc.tile_pool(name="w3pool", bufs=1))
        w3_sb = w3pool.tile([128, KC, D], MM_DT)
        nc.sync.dma_start(out=w3_sb, in_=w3_d.rearrange("(c p) n -> p c n", p=128))

        # ---- ACT-ring DMAs: query side + weights ----
        ident = singles.tile([128, 128], F32)
        make_identity(nc, ident)
        identR = singles.tile([128, 128], MM_DT)
        nc.vector.tensor_copy(identR, ident)

        pw_sb = singles.tile([128, KC, 1], MM_DT)
        nc.scalar.dma_start(out=pw_sb, in_=pw_d.rearrange("(c p) x -> p c x", p=128))
        biasc_sb = singles.tile([1, D], F32)
        nc.scalar.dma_start(out=biasc_sb, in_=biasc_d)
        w2v_sb = singles.tile([1, D], BF16)
        nc.scalar.dma_start(out=w2v_sb, in_=w2v_d)
        ones_f = singles.tile([1, 128], F32)
        nc.vector.memset(ones_f, 1.0)
        ones_r = singles.tile([1, 128], BF16)
        nc.vector.tensor_copy(ones_r, ones_f)

        # persistent per-batch small tensors
        sentT2 = qstuff.tile([128, KC, BPC], MM_DT)     # sentence^T chunks, col b
        snsq2 = qstuff.tile([1, BPC], F32)              # clamped ||sentence||^2
        vb_sb = qstuff.tile([1, BPC, VL], F32)          # log(video_mask) rows
        aug_rhs = []                                    # per batch [1, D] bf16

        # ---------------- Phase A: query side (tiny) -----------------
        q_tiles = []
        for b in range(BPC):
            q_sb = qstuff.tile([QL, D], F32, name=f"q{b}")
            nc.scalar.dma_start(out=q_sb, in_=q_d[b])
            nc.scalar.dma_start(out=vb_sb[:, b, :], in_=vb_d[b : b + 1, :])
            q_tiles.append(q_sb)

        w1_sb = singles.tile([128, KC, D], MM_DT)
        nc.scalar.dma_start(out=w1_sb, in_=w1_d.rearrange("(c p) n -> p c n", p=128))

        for b in range(BPC):
            q_sb = q_tiles[b]
            # Q^T chunks [128, KC, QL]
            qt_sb = qstuff.tile([128, KC, QL], MM_DT, name=f"qt{b}")
            for k in range(KC):
                t_ps = psA.tile([128, QL], F32, tag="tps")
                nc.tensor.transpose(t_ps, q_sb[:, k * 128 : (k + 1) * 128],
                                    ident[:QL, :QL])
                nc.vector.tensor_copy(qt_sb[:, k, :], t_ps)

            # alpha^T = (Q @ pool_w)^T : [1, QL]
            al_ps = psRow.tile([1, QL], F32, tag="row")
            for k in range(KC):
                nc.tensor.matmul(al_ps, pw_sb[:, k, :], qt_sb[:, k, :],
                                 start=(k == 0), stop=(k == KC - 1))
            qb_sb = rows.tile([1, QL], F32)
            nc.scalar.dma_start(out=qb_sb, in_=qb_d[b : b + 1, :])
            alpha_sb = rows.tile([1, QL], F32)
            nc.vector.tensor_add(alpha_sb, al_ps, qb_sb)

            # softmax over the free dim (1 partition)
            mx = rows.tile([1, 1], F32)
            nc.vector.reduce_max(mx, alpha_sb, axis=AX.X)
            asub = rows.tile([1, QL], F32)
            nc.vector.tensor_scalar_sub(asub, alpha_sb, mx)
            aexp = rows.tile([1, QL], F32)
            asum = rows.tile([1, 1], F32)
            nc.scalar.activation(aexp, asub, AF.Exp, accum_out=asum)
            rsum = rows.tile([1, 1], F32)
            nc.vector.reciprocal(rsum, asum)
            alphas_sb = rows.tile([1, QL], F32)
            nc.vector.tensor_scalar_mul(alphas_sb, aexp, rsum)

            # alphas^T : [QL, 1]
            alT_ps = psRow.tile([QL, 1], F32, tag="row")
            nc.tensor.transpose(alT_ps, alphas_sb, ident[:1, :1])
            alphasT_sb = rows.tile([QL, 1], F32)
            nc.vector.tensor_copy(alphasT_sb, alT_ps)

            # sentence = alphas @ Q : [1, D]  (plain fp32, tiny)
            sent_sb = rows.tile([1, D], F32, tag="sent", bufs=1)
            for h in range(2):
                s_ps = psRow.tile([1, 512], F32, tag="row")
                nc.tensor.matmul(s_ps, alphasT_sb, q_sb[:, h * 512 : (h + 1) * 512],
                                 start=True, stop=True)
                nc.vector.tensor_copy(sent_sb[:, h * 512 : (h + 1) * 512], s_ps)

            # ||sentence||^2 clamped
            strash = rows.tile([1, D], F32, tag="strash", bufs=1)
            ssq = rows.tile([1, 1], F32)
            nc.scalar.activation(strash, sent_sb, AF.Square, accum_out=ssq)
            nc.vector.tensor_scalar_max(snsq2[:, b : b + 1], ssq, 1e-16)

            # sentence^T chunks into column b
            for k in range(KC):
                sT_ps = psRow.tile([128, 1], F32, tag="row")
                nc.tensor.transpose(sT_ps, sent_sb[:, k * 128 : (k + 1) * 128],
                                    ident[:1, :1])
                nc.vector.tensor_copy(sentT2[:, k, b : b + 1], sT_ps)

        # bias row per batch: [1, D] = sentence @ W3 + biasc  (bf16 result)
        for b in range(BPC):
            bias_b = qstuff.tile([1, D], BF16, name=f"bias{b}")
            for h in range(2):
                b_ps = psRow.tile([1, 512], F32, tag="row")
                for k in range(KC):
                    nc.tensor.matmul(b_ps, sentT2[:, k, b : b + 1],
                                     w3_sb[:, k, h * 512 : (h + 1) * 512],
                                     start=(k == 0), stop=(k == KC - 1))
                nc.vector.tensor_add(bias_b[:, h * 512 : (h + 1) * 512], b_ps,
                                     biasc_sb[:, h * 512 : (h + 1) * 512])
            aug_rhs.append(bias_b)

        w3ctx.close()  # release W3's SBUF before the heavy phase

        # ---------------- Phase C: video side (heavy) -----------------
        trashp = ctx.enter_context(tc.tile_pool(name="trashp", bufs=2))
        vtpool = ctx.enter_context(tc.tile_pool(name="vtpool", bufs=2))
        opool = ctx.enter_context(tc.tile_pool(name="opool", bufs=3))

        for b in range(BPC):
            for ss in range(NSS):
                vt = vtpool.tile([128, KC, SS], MM_DT)
                vnsq_col = rows.tile([128, 4], F32, tag="vnsqc")
                for s4 in range(4):
                    r0 = ss * SS + s4 * 128
                    if b == 0 and ss == 0:
                        v_sb = pre_v[s4]
                    else:
                        v_sb = vload.tile([128, D], MM_DT, tag="v_sb")
                        nc.sync.dma_start(out=v_sb, in_=v_d[b, r0 : r0 + 128, :])
                    # ||v||^2 per row (accumulate over free dim)
                    vtrash = trashp.tile([128, D], F32, tag="vtrash")
                    nc.scalar.activation(vtrash, v_sb.bitcast(F32), AF.Square,
                                         accum_out=vnsq_col[:, s4 : s4 + 1])
                    # transpose 8 chunks of [128,128] -> vt
                    for g in range(2):
                        t_ps = psA.tile([128, 512], MM_DT, tag="tps")
                        for j in range(4):
                            k = g * 4 + j
                            nc.tensor.transpose(
                                t_ps[:, j * 128 : (j + 1) * 128],
                                v_sb[:, k * 128 : (k + 1) * 128], identR)
                        nc.vector.tensor_copy(
                            vt[:, g * 4 : (g + 1) * 4, s4 * 128 : (s4 + 1) * 128],
                            t_ps.rearrange("p (j c) -> p j c", j=4))

                # vnsq as a row [1, SS] via 4 tiny transposes
                vnr_ps = psRow.tile([1, SS], F32, tag="row")
                for s4 in range(4):
                    nc.tensor.transpose(vnr_ps[:, s4 * 128 : (s4 + 1) * 128],
                                        vnsq_col[:, s4 : s4 + 1], ident)

                # dot = V @ sentence as a row [1, SS]
                dot_ps = psRow.tile([1, SS], F32, tag="row")
                for k in range(KC):
                    nc.tensor.matmul(dot_ps, sentT2[:, k, b : b + 1], vt[:, k, :],
                                     start=(k == 0), stop=(k == KC - 1))

                # sim = dot / sqrt(max(vnsq,eps)*snsq) + log(video_mask)
                t1 = rows.tile([1, SS], F32, tag="t1")
                nc.vector.tensor_scalar(t1, vnr_ps, 1e-16, snsq2[:, b : b + 1],
                                        op0=mybir.AluOpType.max,
                                        op1=mybir.AluOpType.mult)
                t2 = rows.tile([1, SS], F32, tag="t2")
                nc.scalar.activation(t2, t1, AF.Sqrt)
                t3 = rows.tile([1, SS], F32, tag="t3")
                nc.vector.reciprocal(t3, t2)
                t4 = rows.tile([1, SS], F32, tag="t4")
                nc.vector.tensor_mul(t4, dot_ps, t3)
                sim_r = rows.tile([1, SS], BF16, tag="simr")
                nc.vector.tensor_add(sim_r, t4,
                                     vb_sb[:, b, ss * SS : (ss + 1) * SS])

                # main matmul + two rank-1 bf16 augments + relu
                for i in range(4):
                    out_sb = opool.tile([128, D], F32)
                    o_ps = [psOut.tile([128, 512], F32, tag="o_ps", name=f"o_ps_{b}_{ss}_{i}_{h}")
                            for h in range(2)]
                    for k in range(KC):
                        for h in range(2):
                            nc.tensor.matmul(
                                o_ps[h], vt[:, k, i * 128 : (i + 1) * 128],
                                w1_sb[:, k, h * 512 : (h + 1) * 512],
                                start=(k == 0), stop=False)
                    for h in range(2):
                        nc.tensor.matmul(
                            o_ps[h], sim_r[:, i * 128 : (i + 1) * 128],
                            w2v_sb[:, h * 512 : (h + 1) * 512],
                            start=False, stop=False)
                        nc.tensor.matmul(
                            o_ps[h], ones_r,
                            aug_rhs[b][:, h * 512 : (h + 1) * 512],
                            start=False, stop=True)
                        nc.scalar.activation(out_sb[:, h * 512 : (h + 1) * 512],
                                             o_ps[h], AF.Relu)
                    r0 = ss * SS + i * 128
                    nc.scalar.dma_start(out=out_d[b, r0 : r0 + 128, :], in_=out_sb)

    nc.compile()
    return nc


_NC = None
_LAST_RESULTS = None


def _get_program():
    global _NC
    if _NC is None:
        _NC = _build_program()
    return _NC


def kernel(video_features, query_features, video_mask, query_mask,
           sim_w, cor_v_w, cor_q_w, pool_w, mixer_w, mixer_b):
    video_features = np.ascontiguousarray(np.asarray(video_features, dtype=np.float32))
    query_features = np.ascontiguousarray(np.asarray(query_features, dtype=np.float32))
    video_mask = np.asarray(video_mask, dtype=np.float32)
    query_mask = np.asarray(query_mask, dtype=np.float32)
    sim_w = np.asarray(sim_w, dtype=np.float32)
    cor_v_w = np.asarray(cor_v_w, dtype=np.float32)
    cor_q_w = np.asarray(cor_q_w, dtype=np.float32)
    pool_w = np.asarray(pool_w, dtype=np.float32)
    mixer_w = np.asarray(mixer_w, dtype=np.float32)
    mixer_b = np.asarray(mixer_b, dtype=np.float32)

    # host-side folds of the weight-only algebra (O(d^2), negligible)
    W1 = np.ascontiguousarray(mixer_w[0:D])
    W2 = mixer_w[D : 2 * D]
    W3 = np.ascontiguousarray(mixer_w[2 * D : 3 * D])
    W4 = mixer_w[3 * D : 4 * D]
    w2v = (sim_w[:, 0] @ W2).astype(ml_dtypes.bfloat16)[None, :]
    cor_vec = (cor_v_w[0] * cor_q_w[0, 0]).astype(np.float32)
    biasc = (cor_vec @ W4 + mixer_b).astype(np.float32)[None, :]
    qbias = ((1.0 - query_mask) * NEG_INF).astype(np.float32)
    vbias = np.log(video_mask + 1e-45).astype(np.float32)

    nc = _get_program()
    in_maps = []
    for c in range(NCORES):
        sl = slice(c * BPC, (c + 1) * BPC)
        in_maps.append({
            "v": video_features[sl],
            "q": query_features[sl],
            "qb": np.ascontiguousarray(qbias[sl]),
            "vb": np.ascontiguousarray(vbias[sl]),
            "w1": W1,
            "w3": W3,
            "w2v": w2v,
            "biasc": biasc,
            "pw": pool_w,
        })
    res = run_bass_kernel_spmd(nc, in_maps, core_ids=list(range(NCORES)))
    global _LAST_RESULTS
    _LAST_RESULTS = res
    out = np.concatenate([res.results[c]["out"] for c in range(NCORES)], axis=0)
    return out.astype(np.float32, copy=False)


# revision 22
# speedup vs baseline: 1.1502x; 1.0050x over previous
"""Trainium2 Bass kernel for nn_FeatureRefinement.

Reference computation (bs=16, vl=1024, ql=64, d=1024):
    corr = einsum('bqd,bvd->bqv', Q, V); scores = softmax(corr, axis=1)
    corr_matrix = einsum('bqv,qd->bvd', scores, cor_w)     # cor_w constant over q
    sentence    = WeightedPool(Q)                           # (bs, d)
    sim         = cosine(V, sentence) + log(video_mask)     # (bs, vl)
    features    = concat([V, sim*sim_w, sentence_bcast, corr_matrix], -1)
    out         = relu(features @ mixer_w + mixer_b)

Algebraic restructuring (exact up to fp rounding):
  - softmax over q sums to 1  =>  corr_matrix[b,v,:] == cor_v_w*cor_q_w  (constant)
  - sim_features @ W2  == sim[b,v] * (sim_w.T @ W2)        (rank-1)
  - pooled_query @ W3  == sentence[b] @ W3                 (rank-1 per batch)
  so   out = relu(V @ W1 + sim ⊗ w2v + 1 ⊗ (sentence@W3 + cor@W4 + bias))
  The only heavy compute is V @ W1 (4x FLOP reduction) plus O(bs*vl*d)
  vector work for the cosine similarity.

Sharding: data-parallel over batch, 2 batches per core on 8 cores. No
collectives; host scatters inputs / gathers outputs.

DMA rings: V slabs (+W3) stream on the SP ring so the PE's transpose
pipeline starts ~immediately; weights/query/outputs ride the ACT ring.
Matmul dtypes: fp32r (hw runs LOW_HIGH 2-pass, ~1.5e-4 rel err) for the
V@W1 path; bf16 for the small rank-1 augment terms.
"""
import sys

sys.path.insert(0, "/opt/trn_rl_repo")

import numpy as np
import ml_dtypes
from contextlib import ExitStack

import concourse.bass as bass
import concourse.tile as tile
from concourse import bacc, mybir
from concourse.bass_utils import run_bass_kernel_spmd
from concourse.masks import make_identity

F32 = mybir.dt.float32
F32R = mybir.dt.float32r
BF16 = mybir.dt.bfloat16
AF = mybir.ActivationFunctionType
AX = mybir.AxisListType

BS, VL, QL, D = 16, 1024, 64, 1024
NCORES = 8
BPC = BS // NCORES          # batches per core
KC = D // 128               # contraction chunks
SS = 512                    # v-rows per super-slab
NSS = VL // SS              # super-slabs per batch
NEG_INF = -1e30

MM_DT = F32R


def _build_program():
    nc = bacc.Bacc("TRN2", target_bir_lowering=False, debug=False, num_devices=NCORES)

    v_d = nc.dram_tensor("v", [BPC, VL, D], MM_DT, kind="ExternalInput").ap()
    q_d = nc.dram_tensor("q", [BPC, QL, D], F32, kind="ExternalInput").ap()
    qb_d = nc.dram_tensor("qb", [BPC, QL], F32, kind="ExternalInput").ap()
    vb_d = nc.dram_tensor("vb", [BPC, VL], F32, kind="ExternalInput").ap()
    w1_d = nc.dram_tensor("w1", [D, D], MM_DT, kind="ExternalInput").ap()
    w3_d = nc.dram_tensor("w3", [D, D], MM_DT, kind="ExternalInput").ap()
    w2v_d = nc.dram_tensor("w2v", [1, D], BF16, kind="ExternalInput").ap()
    biasc_d = nc.dram_tensor("biasc", [1, D], F32, kind="ExternalInput").ap()
    pw_d = nc.dram_tensor("pw", [D, 1], MM_DT, kind="ExternalInput").ap()
    out_d = nc.dram_tensor("out", [BPC, VL, D], F32, kind="ExternalOutput").ap()

    with tile.TileContext(nc) as tc, ExitStack() as ctx:
        singles = ctx.enter_context(tc.tile_pool(name="singles", bufs=1))
        qstuff = ctx.enter_context(tc.tile_pool(name="qstuff", bufs=1))
        rows = ctx.enter_context(tc.tile_pool(name="rows", bufs=2))
        vload = ctx.enter_context(tc.tile_pool(name="vload", bufs=5))
        psA = ctx.enter_context(tc.tile_pool(name="psA", bufs=2, space="PSUM"))
        psOut = ctx.enter_context(tc.tile_pool(name="psOut", bufs=4, space="PSUM"))
        psRow = ctx.enter_context(tc.tile_pool(name="psRow", bufs=2, space="PSUM"))

        # ---- SP-ring DMAs first: V slabs for (b0, ss0), then W3 ----
        pre_v = []
        for s4 in range(4):
            v_sb = vload.tile([128, D], MM_DT, tag="v_sb")
            nc.sync.dma_start(out=v_sb, in_=v_d[0, s4 * 128 : (s4 + 1) * 128, :])
            pre_v.append(v_sb)

        # ---- ACT-ring DMAs: query side + weights ----
        ident = singles.tile([128, 128], F32)
        make_identity(nc, ident)
        identR = singles.tile([128, 128], MM_DT)
        nc.vector.tensor_copy(identR, ident)

        pw_sb = singles.tile([128, KC, 1], MM_DT)
        nc.scalar.dma_start(out=pw_sb, in_=pw_d.rearrange("(c p) x -> p c x", p=128))
        biasc_sb = singles.tile([1, D], F32)
        nc.scalar.dma_start(out=biasc_sb, in_=biasc_d)
        w2v_sb = singles.tile([1, D], BF16)
        nc.scalar.dma_start(out=w2v_sb, in_=w2v_d)
        ones_f = singles.tile([1, 128], F32)
        nc.vector.memset(ones_f, 1.0)
        ones_r = singles.tile([1, 128], BF16)
        nc.vector.tensor_copy(ones_r, ones_f)

        # persistent per-batch small tensors
        sentT2 = qstuff.tile([128, KC, BPC], MM_DT)     # sentence^T chunks, col b
        snsq2 = qstuff.tile([1, BPC], F32)              # clamped ||sentence||^2
        vb_sb = qstuff.tile([1, BPC, VL], F32)          # log(video_mask) rows
        aug_rhs = []                                    # per batch [1, D] bf16

        # ---------------- Phase A: query side (tiny) -----------------
        q_tiles = []
        for b in range(BPC):
            q_sb = qstuff.tile([QL, D], F32, name=f"q{b}")
            nc.scalar.dma_start(out=q_sb, in_=q_d[b])
            nc.scalar.dma_start(out=vb_sb[:, b, :], in_=vb_d[b : b + 1, :])
            q_tiles.append(q_sb)

        w3ctx = ExitStack()
        w3pool = w3ctx.enter_context(tc.tile_pool(name="w3pool", bufs=1))
        w3_sb = w3pool.tile([128, KC, D], MM_DT)
        nc.scalar.dma_start(out=w3_sb, in_=w3_d.rearrange("(c p) n -> p c n", p=128))

        w1_sb = singles.tile([128, KC, D], MM_DT)
        nc.scalar.dma_start(out=w1_sb, in_=w1_d.rearrange("(c p) n -> p c n", p=128))

        for b in range(BPC):
            q_sb = q_tiles[b]
            # Q^T chunks [128, KC, QL]
            qt_sb = qstuff.tile([128, KC, QL], MM_DT, name=f"qt{b}")
            for k in range(KC):
                t_ps = psA.tile([128, QL], F32, tag="tps")
                nc.tensor.transpose(t_ps, q_sb[:, k * 128 : (k + 1) * 128],
                                    ident[:QL, :QL])
                nc.vector.tensor_copy(qt_sb[:, k, :], t_ps)

            # alpha^T = (Q @ pool_w)^T : [1, QL]
            al_ps = psRow.tile([1, QL], F32, tag="row")
            for k in range(KC):
                nc.tensor.matmul(al_ps, pw_sb[:, k, :], qt_sb[:, k, :],
                                 start=(k == 0), stop=(k == KC - 1))
            qb_sb = rows.tile([1, QL], F32)
            nc.scalar.dma_start(out=qb_sb, in_=qb_d[b : b + 1, :])
            alpha_sb = rows.tile([1, QL], F32)
            nc.vector.tensor_add(alpha_sb, al_ps, qb_sb)

            # softmax over the free dim (1 partition)
            mx = rows.tile([1, 1], F32)
            nc.vector.reduce_max(mx, alpha_sb, axis=AX.X)
            asub = rows.tile([1, QL], F32)
            nc.vector.tensor_scalar_sub(asub, alpha_sb, mx)
            aexp = rows.tile([1, QL], F32)
            asum = rows.tile([1, 1], F32)
            nc.scalar.activation(aexp, asub, AF.Exp, accum_out=asum)
            rsum = rows.tile([1, 1], F32)
            nc.vector.reciprocal(rsum, asum)
            alphas_sb = rows.tile([1, QL], F32)
            nc.vector.tensor_scalar_mul(alphas_sb, aexp, rsum)

            # alphas^T : [QL, 1]
            alT_ps = psRow.tile([QL, 1], F32, tag="row")
            nc.tensor.transpose(alT_ps, alphas_sb, ident[:1, :1])
            alphasT_sb = rows.tile([QL, 1], F32)
            nc.vector.tensor_copy(alphasT_sb, alT_ps)

            # sentence = alphas @ Q : [1, D]  (plain fp32, tiny)
            sent_sb = rows.tile([1, D], F32, tag="sent", bufs=1)
            for h in range(2):
                s_ps = psRow.tile([1, 512], F32, tag="row")
                nc.tensor.matmul(s_ps, alphasT_sb, q_sb[:, h * 512 : (h + 1) * 512],
                                 start=True, stop=True)
                nc.vector.tensor_copy(sent_sb[:, h * 512 : (h + 1) * 512], s_ps)

            # ||sentence||^2 clamped
            strash = rows.tile([1, D], F32, tag="strash", bufs=1)
            ssq = rows.tile([1, 1], F32)
            nc.scalar.activation(strash, sent_sb, AF.Square, accum_out=ssq)
            nc.vector.tensor_scalar_max(snsq2[:, b : b + 1], ssq, 1e-16)

            # sentence^T chunks into column b
            for k in range(KC):
                sT_ps = psRow.tile([128, 1], F32, tag="row")
                nc.tensor.transpose(sT_ps, sent_sb[:, k * 128 : (k + 1) * 128],
                                    ident[:1, :1])
                nc.vector.tensor_copy(sentT2[:, k, b : b + 1], sT_ps)

        # bias row per batch: [1, D] = sentence @ W3 + biasc, split into
        # bf16 hi + lo so two 1-pass bf16 matmuls carry fp32-grade accuracy
        for b in range(BPC):
            bias_f = rows.tile([1, D], F32, tag="biasf", bufs=1)
            for h in range(2):
                b_ps = psRow.tile([1, 512], F32, tag="row")
                for k in range(KC):
                    nc.tensor.matmul(b_ps, sentT2[:, k, b : b + 1],
                                     w3_sb[:, k, h * 512 : (h + 1) * 512],
                                     start=(k == 0), stop=(k == KC - 1))
                nc.vector.tensor_add(bias_f[:, h * 512 : (h + 1) * 512], b_ps,
                                     biasc_sb[:, h * 512 : (h + 1) * 512])
            bias_hi = qstuff.tile([1, D], BF16, name=f"biashi{b}")
            nc.vector.tensor_copy(bias_hi, bias_f)
            bias_lo = qstuff.tile([1, D], BF16, name=f"biaslo{b}")
            nc.vector.tensor_sub(bias_lo, bias_f, bias_hi)
            aug_rhs.append((bias_hi, bias_lo))

        w3ctx.close()  # release W3's SBUF before the heavy phase

        # ---------------- Phase C: video side (heavy) -----------------
        trashp = ctx.enter_context(tc.tile_pool(name="trashp", bufs=2))
        vtpool = ctx.enter_context(tc.tile_pool(name="vtpool", bufs=2))
        opool = ctx.enter_context(tc.tile_pool(name="opool", bufs=3))

        for b in range(BPC):
            for ss in range(NSS):
                vt = vtpool.tile([128, KC, SS], MM_DT)
                vnsq_col = rows.tile([128, 4], F32, tag="vnsqc")
                for s4 in range(4):
                    r0 = ss * SS + s4 * 128
                    if b == 0 and ss == 0:
                        v_sb = pre_v[s4]
                    else:
                        v_sb = vload.tile([128, D], MM_DT, tag="v_sb")
                        nc.sync.dma_start(out=v_sb, in_=v_d[b, r0 : r0 + 128, :])
                    # ||v||^2 per row (accumulate over free dim)
                    vtrash = trashp.tile([128, D], F32, tag="vtrash")
                    nc.scalar.activation(vtrash, v_sb.bitcast(F32), AF.Square,
                                         accum_out=vnsq_col[:, s4 : s4 + 1])
                    # transpose 8 chunks of [128,128] -> vt
                    for g in range(2):
                        t_ps = psA.tile([128, 512], MM_DT, tag="tps")
                        for j in range(4):
                            k = g * 4 + j
                            nc.tensor.transpose(
                                t_ps[:, j * 128 : (j + 1) * 128],
                                v_sb[:, k * 128 : (k + 1) * 128], identR)
                        nc.vector.tensor_copy(
                            vt[:, g * 4 : (g + 1) * 4, s4 * 128 : (s4 + 1) * 128],
                            t_ps.rearrange("p (j c) -> p j c", j=4))

                # vnsq as a row [1, SS] via 4 tiny transposes
                vnr_ps = psRow.tile([1, SS], F32, tag="row")
                for s4 in range(4):
                    nc.tensor.transpose(vnr_ps[:, s4 * 128 : (s4 + 1) * 128],
                                        vnsq_col[:, s4 : s4 + 1], ident)

                # dot = V @ sentence as a row [1, SS]
                dot_ps = psRow.tile([1, SS], F32, tag="row")
                for k in range(KC):
                    nc.tensor.matmul(dot_ps, sentT2[:, k, b : b + 1], vt[:, k, :],
                                     start=(k == 0), stop=(k == KC - 1))

                # sim = dot / sqrt(max(vnsq,eps)*snsq) + log(video_mask)
                t1 = rows.tile([1, SS], F32, tag="t1")
                nc.vector.tensor_scalar(t1, vnr_ps, 1e-16, snsq2[:, b : b + 1],
                                        op0=mybir.AluOpType.max,
                                        op1=mybir.AluOpType.mult)
                t3 = rows.tile([1, SS], F32, tag="t3")
                nc.scalar.activation(t3, t1, AF.Abs_reciprocal_sqrt)
                t4 = rows.tile([1, SS], F32, tag="t4")
                nc.vector.tensor_mul(t4, dot_ps, t3)
                sim_r = rows.tile([1, SS], BF16, tag="simr")
                nc.vector.tensor_add(sim_r, t4,
                                     vb_sb[:, b, ss * SS : (ss + 1) * SS])

                # main matmul + two rank-1 bf16 augments + relu
                for i in range(4):
                    out_sb = opool.tile([128, D], F32)
                    o_ps = [psOut.tile([128, 512], F32, tag="o_ps", name=f"o_ps_{b}_{ss}_{i}_{h}")
                            for h in range(2)]
                    for k in range(KC):
                        for h in range(2):
                            nc.tensor.matmul(
                                o_ps[h], vt[:, k, i * 128 : (i + 1) * 128],
                                w1_sb[:, k, h * 512 : (h + 1) * 512],
                                start=(k == 0), stop=False)
                    bias_hi, bias_lo = aug_rhs[b]
                    for h in range(2):
                        nc.tensor.matmul(
                            o_ps[h], sim_r[:, i * 128 : (i + 1) * 128],
                            w2v_sb[:, h * 512 : (h + 1) * 512],
                            start=False, stop=False)
                        nc.tensor.matmul(
                            o_ps[h], ones_r,
                            bias_hi[:, h * 512 : (h + 1) * 512],
                            start=False, stop=False)
                        nc.tensor.matmul(
                            o_ps[h], ones_r,
                            bias_lo[:, h * 512 : (h + 1) * 512],
                            start=False, stop=True)
                        nc.scalar.activation(out_sb[:, h * 512 : (h + 1) * 512],
                                             o_ps[h], AF.Relu)
                    r0 = ss * SS + i * 128
                    nc.scalar.dma_start(out=out_d[b, r0 : r0 + 128, :], in_=out_sb)

    nc.compile()
    return nc


_NC = None
_LAST_RESULTS = None


def _get_program():
    global _NC
    if _NC is None:
        _NC = _build_program()
    return _NC


def kernel(video_features, query_features, video_mask, query_mask,
           sim_w, cor_v_w, cor_q_w, pool_w, mixer_w, mixer_b):
    video_features = np.ascontiguousarray(np.asarray(video_features, dtype=np.float32))
    query_features = np.ascontiguousarray(np.asarray(query_features, dtype=np.float32))
    video_mask = np.asarray(video_mask, dtype=np.float32)
    query_mask = np.asarray(query_mask, dtype=np.float32)
    sim_w = np.asarray(sim_w, dtype=np.float32)
    cor_v_w = np.asarray(cor_v_w, dtype=np.float32)
    cor_q_w = np.asarray(cor_q_w, dtype=np.float32)
    pool_w = np.asarray(pool_w, dtype=np.float32)
    mixer_w = np.asarray(mixer_w, dtype=np.float32)
    mixer_b = np.asarray(mixer_b, dtype=np.float32)

    # host-side folds of the weight-only algebra (O(d^2), negligible)
    W1 = np.ascontiguousarray(mixer_w[0:D])
    W2 = mixer_w[D : 2 * D]
    W3 = np.ascontiguousarray(mixer_w[2 * D : 3 * D])
    W4 = mixer_w[3 * D : 4 * D]
    w2v = (sim_w[:, 0] @ W2).astype(ml_dtypes.bfloat16)[None, :]
    cor_vec = (cor_v_w[0] * cor_q_w[0, 0]).astype(np.float32)
    biasc = (cor_vec @ W4 + mixer_b).astype(np.float32)[None, :]
    qbias = ((1.0 - query_mask) * NEG_INF).astype(np.float32)
    vbias = np.log(video_mask + 1e-45).astype(np.float32)

    nc = _get_program()
    in_maps = []
    for c in range(NCORES):
        sl = slice(c * BPC, (c + 1) * BPC)
        in_maps.append({
            "v": video_features[sl],
            "q": query_features[sl],
            "qb": np.ascontiguousarray(qbias[sl]),
            "vb": np.ascontiguousarray(vbias[sl]),
            "w1": W1,
            "w3": W3,
            "w2v": w2v,
            "biasc": biasc,
            "pw": pool_w,
        })
    res = run_bass_kernel_spmd(nc, in_maps, core_ids=list(range(NCORES)))
    global _LAST_RESULTS
    _LAST_RESULTS = res
    out = np.concatenate([res.results[c]["out"] for c in range(NCORES)], axis=0)
    return out.astype(np.float32, copy=False)
